# revision 1
# baseline (speedup 1.0000x reference)
"""DechirpSTFT Trainium2 kernel (8 NeuronCores), radix-4 bf16 design.

Math: out[d,b,w,:] = FFT_1024(chirp * resample_d(hann * window(x[b], w)))

Factorization per (d, b), K = 1024 = 4 x 256 (DIT, n = 4*n1 + n2):
  - window + hann + linear-interp resample -> banded matrix G_d applied by
    TensorE to x held in SBUF as [128, 4104] bf16 (window = stride-4 column
    slice; hop 512 = 4 cols of 128).  G's columns emit y directly in radix-4
    order: y-tile (n2, t4) holds y[4*(128*t4 + p) + n2]  (8 tiles, 5 source
    slots each on the 64-sample-shifted x grid).
  - stage-1: four 256-point complex DFTs
      M_{n2}[n1,k1] = chirp[4n1+n2] * W256^{n1 k1} * W1024^{n2 k1}
    (chirp + twiddles folded in), TensorE, contraction over n1 (2 k-tiles).
  - radix-4 combine (twiddle-free): with A± = V0±V2, B± = V1±V3:
      Y[k1]      = A+ + B+        Y[k1+512] = A+ - B+
      Y[k1+256]  = A- - i B-      Y[k1+768] = A- + i B-
    V0/V1 evacuated PSUM->SBUF f32 by ACT; partials on DVE (one PSUM
    operand); outputs on DVE in bf16 (2x packed mode), written as separate
    re/im planes for contiguous bf16 output DMA (host interleaves).
All matmuls bf16 (1 cyc/row, FWL weight loads); overall rel err ~4e-3.
Each core owns 2 of the 16 chirp rates.
"""

import numpy as np
import ml_dtypes

K = 1024
HOP = 512
CHIRP_A = 0.5
NB = 2
NX = 524288
W = (NX - K) // HOP + 1          # 1023
D = 16
NCORES = 8
DLOC = D // NCORES               # 2 chirp rates per core
WT = 512                          # windows per chunk (matmul moving dim)
NWC = 2                           # ceil(1023/512)
NSLOT = 5                         # interp source tiles per y-tile
XCOLS = 4104                      # 4096 cols + pad so window 1022 reads zeros

_NC_CACHE = {}
_LAST_RESULTS = {}
_REPEAT = 1  # >1: wrap body in a device-side loop (timing experiments only)
_VARIANT = "full"  # timing-only: full | nodma | notail


def _host_tables_all(dlnf):
    """(16,) -> lo (D,K) int32, frac (D,K) f32.  Computed with jax on CPU,
    bit-exactly mirroring reference.py's fp32 pipeline."""
    import jax
    import jax.numpy as jnp

    cpu = jax.devices("cpu")[0]
    with jax.default_device(cpu):
        betas = 2.0 * jnp.asarray(np.asarray(dlnf, dtype=np.float32))
        safe = jnp.abs(betas) < 1e-8
        bs = jnp.where(safe, jnp.float32(1e-8), betas)
        tau = jnp.linspace(0.0, 1.0, K, dtype=jnp.float32)
        t_src = 2.0 / bs[:, None] * jnp.log1p(
            tau[None, :] * (jnp.exp(bs)[:, None] - 1.0)) - 1.0
        identity = jnp.linspace(-1.0, 1.0, K, dtype=jnp.float32)
        t_src = jnp.where(safe[:, None], identity[None, :], t_src)
        idx = (t_src + 1.0) * 0.5 * (K - 1)
        lo = jnp.clip(idx.astype(jnp.int32), 0, K - 2)
        frac = idx - lo.astype(idx.dtype)
    return np.asarray(lo), np.asarray(frac).astype(np.float32)


def _build_g(lo_pair, frac_pair):
    """Interp stationaries, packed [128, DLOC*4*2*NSLOT*128] bf16.
    Col block ((d2*4+n2)*2+t4)*NSLOT+s holds G[q, p]:
    src j = 128*(4*t4+s) + q - 64  ->  n = 4*(128*t4+p) + n2."""
    hann = (0.5 * (1.0 - np.cos(2.0 * np.pi * np.arange(K) / K))).astype(np.float32)
    g = np.zeros((128, DLOC * 4 * 2 * NSLOT * 128), dtype=np.float32)
    nn = np.arange(K)
    n2a, n1 = nn & 3, nn >> 2
    t4a, pa = n1 >> 7, n1 & 127
    for d2 in range(DLOC):
        lo = lo_pair[d2]
        frac = frac_pair[d2]
        alpha = ((1.0 - frac) * hann[lo]).astype(np.float32)
        beta = (frac * hann[lo + 1]).astype(np.float32)
        for j, val in ((lo, alpha), (lo + 1, beta)):
            m, q = (j + 64) >> 7, (j + 64) & 127
            s = m - 4 * t4a
            if not np.all((s >= 0) & (s < NSLOT)):
                raise ValueError("interp band exceeds the 5 source-tile slots")
            flat = ((d2 * 4 + n2a) * 2 + t4a) * NSLOT + s
            np.add.at(g, (q, flat * 128 + pa), val)
    return g.astype(ml_dtypes.bfloat16)


def _build_m1():
    """Stage-1 DFT stationaries [128, 4*2*2*2*128] bf16 (d-independent).
    Col block ((n2*2+pl)*2+kt)*2+mc holds M[q, c]: n1=128*kt+q, k1=128*mc+c."""
    t_norm = np.linspace(-1.0, 1.0, K).astype(np.float64)
    chirp = np.exp(-1j * CHIRP_A * t_norm ** 2)
    m1 = np.zeros((128, 4 * 2 * 2 * 2 * 128), dtype=np.float32)
    n1g = np.arange(256)
    k1g = np.arange(256)
    for n2 in range(4):
        M = (chirp[4 * n1g + n2][:, None]
             * np.exp(-2j * np.pi * np.outer(n1g, k1g) / 256)
             * np.exp(-2j * np.pi * n2 * k1g / K)[None, :])
        for pl in range(2):
            plane = (M.real if pl == 0 else M.imag).astype(np.float32)
            for kt in range(2):
                for mc in range(2):
                    flat = ((n2 * 2 + pl) * 2 + kt) * 2 + mc
                    m1[:, flat * 128:(flat + 1) * 128] = \
                        plane[128 * kt:128 * kt + 128, 128 * mc:128 * mc + 128]
    return m1.astype(ml_dtypes.bfloat16)


def _build_program():
    import concourse.bacc as bacc
    import concourse.mybir as mybir
    from concourse.tile import TileContext

    f32 = mybir.dt.float32
    bf16 = mybir.dt.bfloat16

    nc = bacc.Bacc("TRN2", target_bir_lowering=False, debug=False,
                   num_devices=NCORES)
    # phase-split layout: xT[b, q, ph, c4] = x[b, 128*(4*c4 + ph) + q - 64]
    # so every interp moving slice (window stride 4 cols) is CONTIGUOUS
    xT = nc.dram_tensor("xT", [NB, 128, 4, XCOLS // 4], bf16,
                        kind="ExternalInput")
    g = nc.dram_tensor("g", [128, DLOC * 4 * 2 * NSLOT * 128], bf16,
                       kind="ExternalInput")
    m1 = nc.dram_tensor("m1", [128, 4 * 2 * 2 * 2 * 128], bf16,
                        kind="ExternalInput")
    # k decomposed as 256*j + 128*mc + p -> dims [mc, p, j] so one DMA per
    # (mc, plane) covers all 4 j-tiles (HWDGE cost is fixed per DMA)
    out_re = nc.dram_tensor("out_re", [DLOC, NB, 2, 128, 4, W], bf16,
                            kind="ExternalOutput")
    out_im = nc.dram_tensor("out_im", [DLOC, NB, 2, 128, 4, W], bf16,
                            kind="ExternalOutput")

    def gcol(d2, n2, t4, s):
        flat = ((d2 * 4 + n2) * 2 + t4) * NSLOT + s
        return slice(flat * 128, (flat + 1) * 128)

    def m1col(n2, pl, kt, mc):
        flat = ((n2 * 2 + pl) * 2 + kt) * 2 + mc
        return slice(flat * 128, (flat + 1) * 128)

    with TileContext(nc) as tc:
        with (
            tc.tile_pool(name="resident", bufs=1) as rp,
            tc.tile_pool(name="ysb", bufs=24) as yp,
            tc.tile_pool(name="vsb", bufs=8) as sp,
            tc.tile_pool(name="psb", bufs=6) as pp,
            tc.tile_pool(name="osb", bufs=6) as op,
            tc.tile_pool(name="py", bufs=2, space="PSUM") as pyp,
            tc.tile_pool(name="pv", bufs=3, space="PSUM") as pvp,
        ):
            # resident loads split across both HWDGE queues, ordered so the
            # (sub-chunked, 256-window) first iteration's operands land first:
            #   sync: x0[first sub-chunk] -> m1 -> x0[rest] -> x1
            #   ACT : g(d2=0, n2=0,1) -> g(d2=0, n2=2,3) -> g(d2=1)
            gcols = DLOC * 4 * 2 * NSLOT * 128
            gq = gcols // 4
            g_sb = rp.tile([128, gcols], bf16, tag="g")
            for qq in range(4):
                nc.scalar.dma_start(out=g_sb[:, qq * gq:(qq + 1) * gq],
                                    in_=g[:, qq * gq:(qq + 1) * gq])
            xt_sb = []
            NP4 = XCOLS // 4                     # cols per phase plane (1026)
            cut0 = 258                           # c4 cols for first 256 windows
            cut = 514                            # c4 cols for chunk wc=0
            for b in range(NB):
                # SBUF layout [128, ph*NP4 + c4]
                xb = rp.tile([128, XCOLS], bf16, tag=f"x{b}")
                xbr = xb[:, :].rearrange("p (ph c) -> p ph c", ph=4)
                if b == 0:
                    nc.sync.dma_start(out=xbr[:, :, 0:cut0],
                                      in_=xT[b, :, :, 0:cut0])
                    m1_sb = rp.tile([128, 4 * 2 * 2 * 2 * 128], bf16, tag="m1")
                    nc.sync.dma_start(out=m1_sb[:, :], in_=m1[:, :])
                    nc.sync.dma_start(out=xbr[:, :, cut0:cut],
                                      in_=xT[b, :, :, cut0:cut])
                else:
                    nc.sync.dma_start(out=xbr[:, :, 0:cut],
                                      in_=xT[b, :, :, 0:cut])
                nc.sync.dma_start(out=xbr[:, :, cut:], in_=xT[b, :, :, cut:])
                xt_sb.append(xb)

            def emit_interp_pair(d2, b, w0, wt, n2):
                """Both t4 y-tiles of one n2, matmuls interleaved across the
                two PSUM banks so accumulation-group turnarounds overlap."""
                NP4 = XCOLS // 4
                py0 = pyp.tile([128, WT], f32, tag="py")
                py1 = pyp.tile([128, WT], f32, tag="py")
                pys = [py0, py1]
                for s in range(NSLOT):
                    for t4 in range(2):
                        m = 4 * t4 + s
                        col = (m & 3) * NP4 + w0 + (m >> 2)
                        rhs = xt_sb[b][:, col:col + wt]
                        nc.tensor.matmul(
                            pys[t4][:, 0:wt], g_sb[:, gcol(d2, n2, t4, s)],
                            rhs, start=(s == 0), stop=(s == NSLOT - 1))
                row = []
                for t4 in range(2):
                    ysb = yp.tile([128, WT], bf16, tag="y")
                    nc.scalar.copy(ysb[:, 0:wt], pys[t4][:, 0:wt])
                    row.append(ysb)
                return row

            def emit_s1_group(d2, b, w0, wt, mc, pl, ytiles):
                """Stage-1 (mc, pl): 8 matmuls into 2 PSUM pair tiles
                (V0|V1), (V2|V3) + 2 ACT evacs + 4 DVE partials (bf16 2x)."""
                svs = []
                for pr in range(2):
                    pvt = pvp.tile([128, 2 * WT], f32, tag="pv")
                    # interleave the two halves (different PSUM banks) so
                    # accumulation-group turnarounds overlap
                    for kt in range(2):
                        for half in range(2):
                            n2 = 2 * pr + half
                            dst = pvt[:, half * WT:half * WT + wt]
                            nc.tensor.matmul(
                                dst,
                                m1_sb[:, m1col(n2, pl, kt, mc)],
                                ytiles[n2][kt][:, 0:wt],
                                start=(kt == 0), stop=(kt == 1))
                    if _VARIANT == "notail":
                        continue
                    sv = sp.tile([128, 2 * WT], bf16, tag="sv")
                    src = pvt[:, :].rearrange("p (h w) -> p h w", h=2)[:, :, 0:wt]
                    dst = sv[:, :].rearrange("p (h w) -> p h w", h=2)[:, :, 0:wt]
                    nc.scalar.copy(dst, src)
                    svs.append(sv)
                if _VARIANT == "notail":
                    return None
                tdt = f32 if _VARIANT == "tail32" else bf16
                v0 = svs[0][:, 0:wt]
                v1 = svs[0][:, WT:WT + wt]
                v2 = svs[1][:, 0:wt]
                v3 = svs[1][:, WT:WT + wt]
                ap = pp.tile([128, WT], tdt, tag="ap")
                nc.vector.tensor_add(ap[:, 0:wt], v0, v2)
                am = pp.tile([128, WT], tdt, tag="am")
                nc.vector.tensor_sub(am[:, 0:wt], v0, v2)
                bp = pp.tile([128, WT], tdt, tag="bp")
                nc.vector.tensor_add(bp[:, 0:wt], v1, v3)
                bm = pp.tile([128, WT], tdt, tag="bm")
                nc.vector.tensor_sub(bm[:, 0:wt], v1, v3)
                return (ap, am, bp, bm)

            def emit_outputs(d2, b, w0, wt, mc, pab0, pab1):
                """Radix-4 combine for one k1-tile mc: 8 DVE ops + 8 DMAs."""
                if _VARIANT == "notail":
                    return
                wn = min(wt, W - w0)
                (apr, amr, bpr, bmr) = pab0
                (api, ami, bpi, bmi) = pab1
                # j: Y0=A++B+, Y2=A+-B+, Y1=A--iB-, Y3=A-+iB-
                for pli, (dram, tag, specs) in enumerate((
                    (out_re, "or", [(0, apr, bpr, "add"), (1, amr, bmi, "add"),
                                    (2, apr, bpr, "sub"), (3, amr, bmi, "sub")]),
                    (out_im, "oi", [(0, api, bpi, "add"), (1, ami, bmr, "sub"),
                                    (2, api, bpi, "sub"), (3, ami, bmr, "add")]),
                )):
                    ot = op.tile([128, 4 * WT], bf16, tag=tag)
                    for j, in0, in1, kind in specs:
                        dst = ot[:, j * WT:j * WT + wt]
                        if kind == "add":
                            nc.vector.tensor_add(dst, in0[:, 0:wt],
                                                 in1[:, 0:wt])
                        else:
                            nc.vector.tensor_sub(dst, in0[:, 0:wt],
                                                 in1[:, 0:wt])
                    if _VARIANT != "nodma":
                        # alternate HWDGE queues: re plane on SP, im on ACT
                        eng = nc.sync
                        eng.dma_start(
                            out=dram[d2, b, mc, :, :, w0:w0 + wn],
                            in_=ot[:, :].rearrange(
                                "p (j w) -> p j w", j=4)[:, :, 0:wn])

            import contextlib
            import os as _os
            _hints = ()
            if _os.environ.get("LOOP_HINTS"):
                _hints = (mybir.EngineType.PE, mybir.EngineType.Activation,
                          mybir.EngineType.DVE, mybir.EngineType.SP)
            rep_ctx = (tc.For_i(0, _REPEAT, 1, hint_engines=_hints)
                       if _REPEAT > 1 else contextlib.nullcontext())
            with rep_ctx:
                # software pipeline, interleaved at (mc, pl)-group granularity:
                # each stage-1 group of unit i is followed by 2 interp tiles of
                # unit i+1, keeping the ACT evac FIFO aligned with PE's needs
                # (a y-evac is always <= 2 copies behind the matmul needing it).
                # First/last iterations are split into 256-window sub-units so
                # the pipeline fills fast and drains while PE still works.
                units = [(d2, b, WT * wc, WT) for d2 in range(DLOC)
                         for b in range(NB) for wc in range(NWC)]
                if _VARIANT == "interponly":
                    for it in units:
                        for n2 in range(4):
                            emit_interp_pair(*it, n2)
                    units = []
                    ycur = None
                else:
                    ycur = [emit_interp_pair(*units[0], n2) for n2 in range(4)]
                for i, it in enumerate(units):
                    nxt = units[i + 1] if i + 1 < len(units) else None
                    ynxt = [None] * 4 if nxt else None
                    pab = {}
                    for gidx, (mc, pl) in enumerate(
                            [(0, 0), (0, 1), (1, 0), (1, 1)]):
                        pab[(mc, pl)] = emit_s1_group(*it, mc, pl, ycur)
                        if nxt is not None:
                            ynxt[gidx] = emit_interp_pair(*nxt, gidx)
                        if pl == 1:
                            emit_outputs(*it, mc, pab[(mc, 0)], pab[(mc, 1)])
                    ycur = ynxt
    nc.compile()
    return nc


def _host_prep(x, dlnf):
    x = np.ascontiguousarray(np.asarray(x, dtype=np.float32))
    dlnf = np.asarray(dlnf, dtype=np.float32)
    # x shifted by -64 into partition-interleaved, phase-split layout:
    # xT[b, q, ph, c4] = x[b, 128*(4*c4 + ph) + q - 64]  (zeros outside [0,NX))
    xs = np.zeros((NB, XCOLS * 128), dtype=np.float32)
    xs[:, 64:64 + NX] = x
    xT = np.transpose(xs.reshape(NB, XCOLS, 128), (0, 2, 1))  # [b, q, c]
    xT = xT.reshape(NB, 128, XCOLS // 4, 4).transpose(0, 1, 3, 2)
    xT = np.ascontiguousarray(xT).astype(ml_dtypes.bfloat16)
    m1 = _build_m1()
    lo_all, frac_all = _host_tables_all(dlnf)
    in_maps = []
    for c in range(NCORES):
        gc_ = _build_g(lo_all[DLOC * c: DLOC * (c + 1)],
                       frac_all[DLOC * c: DLOC * (c + 1)])
        in_maps.append({"xT": xT, "g": gc_, "m1": m1})
    return in_maps


def kernel(x, dlnf):
    from concourse.bass_utils import run_bass_kernel_spmd

    in_maps = _host_prep(x, dlnf)
    if "nc" not in _NC_CACHE:
        _NC_CACHE["nc"] = _build_program()
    nc = _NC_CACHE["nc"]
    res = run_bass_kernel_spmd(nc, in_maps, core_ids=list(range(NCORES)))
    _LAST_RESULTS["res"] = res
    outs = []
    for c in range(NCORES):
        # [DLOC, NB, mc, p, j, W] -> k = 256 j + 128 mc + p
        cplx = np.empty((DLOC, NB, K, W), dtype=np.complex64)
        for part, o in (
            (cplx.real, res.results[c]["out_re"]),
            (cplx.imag, res.results[c]["out_im"]),
        ):
            part[...] = np.transpose(
                o.astype(np.float32), (0, 1, 4, 2, 3, 5)).reshape(
                    DLOC, NB, K, W)
        outs.append(np.transpose(cplx, (0, 1, 3, 2)))       # -> [DLOC,NB,W,K]
    return np.concatenate(outs, axis=0)



# revision 2
# speedup vs baseline: 68633.7018x; 68633.7018x over previous
"""DechirpSTFT Trainium2 kernel (8 NeuronCores), col-tiled radix-4 design.

Math: out[d,b,w,:] = FFT_1024(chirp * resample_d(hann * window(x[b], w)))

Factorization per (d, b), K = 1024 = 4 x 256 (DIT, n = 4*n1 + n2):
  - interp (window + hann + linear resample) via COL-TILED matmuls:
    radix y-tile (n2, t4) [128 part = n1-128t4, w] is built from 8
    tile_position matmuls: strip h (out partitions 32h..32h+32, from
    natural 128-sample block m = 4t4+h) x 2 source slots s (the band is
    only +-13 samples, so 2 slots of 128 on the 64-shifted x grid cover
    it).  The 4 strips run concurrently on the PE's 32-col sub-arrays,
    so the 8 MMs cost ~2 full-matmul spans instead of 5 (vs the 5-slot
    full-width formulation).
  - stage-1: four 256-point complex DFTs
      M_{n2}[n1,k1] = chirp[4n1+n2] * W256^{n1 k1} * W1024^{n2 k1}
    (chirp + twiddles folded in), TensorE, contraction over n1 (2 k-tiles),
    written into pv tiles [128, re|im] per (n2, mc).
  - radix-4 combine (twiddle-free): with A+- = V0+-V2, B+- = V1+-V3:
      Y[k1]      = A+ + B+        Y[k1+512] = A+ - B+
      Y[k1+256]  = A- - i B-      Y[k1+768] = A- + i B-
    pv evacuated PSUM->SBUF bf16 by ACT/DVE (re|im paired, FD=1024);
    partials on DVE (paired); combines split DVE/Pool; outputs written
    as separate re/im planes for contiguous bf16 output DMA (host
    interleaves).
All matmuls bf16; overall rel err ~4e-3.  Each core owns 2 chirp rates.
"""

import numpy as np
import ml_dtypes

K = 1024
HOP = 512
CHIRP_A = 0.5
NB = 2
NX = 524288
W = (NX - K) // HOP + 1          # 1023
D = 16
NCORES = 8
DLOC = D // NCORES               # 2 chirp rates per core
WT = 512                          # windows per chunk (matmul moving dim)
NWC = 2                           # ceil(1023/512)
XCOLS = 4104                      # 4096 cols + pad so window 1022 reads zeros
NP4 = XCOLS // 4                  # cols per phase plane (1026)

_NC_CACHE = {}
_LAST_RESULTS = {}
_REPEAT = 1  # >1: wrap body in a device-side loop (timing experiments only)
_VARIANT = "full"  # timing-only: full | nodma | notail | interponly

# tail engine assignment knobs (tuned against TimelineSim)
#   sv evac engine per (mc, n2): 'A' = ACT, 'V' = DVE
_SV_ENG = {(0, 0): "A", (0, 1): "A", (0, 2): "A", (0, 3): "V",
           (1, 0): "A", (1, 1): "A", (1, 2): "V", (1, 3): "V"}
#   combine engine per (j-kind): j02 pairs on DVE, j13 singles on Pool
_J02_ENG = "V"
_J13_ENG = "P"


def _host_tables_all(dlnf):
    """(16,) -> lo (D,K) int32, frac (D,K) f32.  Computed with jax on CPU,
    bit-exactly mirroring reference.py's fp32 pipeline."""
    import jax
    import jax.numpy as jnp

    cpu = jax.devices("cpu")[0]
    with jax.default_device(cpu):
        betas = 2.0 * jnp.asarray(np.asarray(dlnf, dtype=np.float32))
        safe = jnp.abs(betas) < 1e-8
        bs = jnp.where(safe, jnp.float32(1e-8), betas)
        tau = jnp.linspace(0.0, 1.0, K, dtype=jnp.float32)
        t_src = 2.0 / bs[:, None] * jnp.log1p(
            tau[None, :] * (jnp.exp(bs)[:, None] - 1.0)) - 1.0
        identity = jnp.linspace(-1.0, 1.0, K, dtype=jnp.float32)
        t_src = jnp.where(safe[:, None], identity[None, :], t_src)
        idx = (t_src + 1.0) * 0.5 * (K - 1)
        lo = jnp.clip(idx.astype(jnp.int32), 0, K - 2)
        frac = idx - lo.astype(idx.dtype)
    return np.asarray(lo), np.asarray(frac).astype(np.float32)


def _build_g(lo_pair, frac_pair):
    """Interp stationaries [128, DLOC*8*2*128] bf16.
    Block (d2*8 + m)*2 + s holds G[q, 32*n2 + j]:
    src sample 128*(m+s) + q - 64  ->  output n = 128*m + 4*j + n2."""
    hann = (0.5 * (1.0 - np.cos(2.0 * np.pi * np.arange(K) / K))).astype(np.float32)
    g = np.zeros((128, DLOC * 8 * 2 * 128), dtype=np.float32)
    nn = np.arange(K)
    mt = nn >> 7
    chat = 32 * (nn & 3) + ((nn >> 2) & 31)
    for d2 in range(DLOC):
        lo = lo_pair[d2]
        frac = frac_pair[d2]
        alpha = ((1.0 - frac) * hann[lo]).astype(np.float32)
        beta = (frac * hann[lo + 1]).astype(np.float32)
        for j, val in ((lo, alpha), (lo + 1, beta)):
            m2, q = (j + 64) >> 7, (j + 64) & 127
            s = m2 - mt
            if not np.all((s >= 0) & (s < 2)):
                raise ValueError("interp band exceeds the 2 source-tile slots")
            flat = (d2 * 8 + mt) * 2 + s
            np.add.at(g, (q, flat * 128 + chat), val)
    return g.astype(ml_dtypes.bfloat16)


def _build_m1():
    """Stage-1 DFT stationaries [128, 4*2*2*2*128] bf16 (d-independent).
    Col block ((n2*2+pl)*2+kt)*2+mc holds M[q, c]: n1=128*kt+q, k1=128*mc+c."""
    t_norm = np.linspace(-1.0, 1.0, K).astype(np.float64)
    chirp = np.exp(-1j * CHIRP_A * t_norm ** 2)
    m1 = np.zeros((128, 4 * 2 * 2 * 2 * 128), dtype=np.float32)
    n1g = np.arange(256)
    k1g = np.arange(256)
    for n2 in range(4):
        M = (chirp[4 * n1g + n2][:, None]
             * np.exp(-2j * np.pi * np.outer(n1g, k1g) / 256)
             * np.exp(-2j * np.pi * n2 * k1g / K)[None, :])
        for pl in range(2):
            plane = (M.real if pl == 0 else M.imag).astype(np.float32)
            for kt in range(2):
                for mc in range(2):
                    flat = ((n2 * 2 + pl) * 2 + kt) * 2 + mc
                    m1[:, flat * 128:(flat + 1) * 128] = \
                        plane[128 * kt:128 * kt + 128, 128 * mc:128 * mc + 128]
    return m1.astype(ml_dtypes.bfloat16)


def _build_program():
    import concourse.bacc as bacc
    import concourse.mybir as mybir
    from concourse.tile import TileContext

    f32 = mybir.dt.float32
    bf16 = mybir.dt.bfloat16

    nc = bacc.Bacc("TRN2", target_bir_lowering=False, debug=False,
                   num_devices=NCORES)
    # phase-split layout: xT[b, q, ph, c4] = x[b, 128*(4*c4 + ph) + q - 64]
    # so every interp moving slice (window stride 4 cols) is CONTIGUOUS
    xT = nc.dram_tensor("xT", [NB, 128, 4, NP4], bf16, kind="ExternalInput")
    g = nc.dram_tensor("g", [128, DLOC * 8 * 2 * 128], bf16,
                       kind="ExternalInput")
    m1 = nc.dram_tensor("m1", [128, 4 * 2 * 2 * 2 * 128], bf16,
                        kind="ExternalInput")
    # k decomposed as 256*j + 128*mc + p -> dims [mc, p, j] so one DMA per
    # (mc, plane) covers all 4 j-tiles (HWDGE cost is fixed per DMA)
    out_re = nc.dram_tensor("out_re", [DLOC, NB, 2, 128, 4, W], bf16,
                            kind="ExternalOutput")
    out_im = nc.dram_tensor("out_im", [DLOC, NB, 2, 128, 4, W], bf16,
                            kind="ExternalOutput")

    def gcol(d2, m, s, n2):
        flat = (d2 * 8 + m) * 2 + s
        return slice(flat * 128 + 32 * n2, flat * 128 + 32 * n2 + 32)

    def m1col(n2, pl, kt, mc):
        flat = ((n2 * 2 + pl) * 2 + kt) * 2 + mc
        return slice(flat * 128, (flat + 1) * 128)

    with TileContext(nc) as tc:
        with (
            tc.tile_pool(name="resident", bufs=1) as rp,
            tc.tile_pool(name="ysb", bufs=18) as yp,
            tc.tile_pool(name="svb", bufs=10) as sp,
            tc.tile_pool(name="pab", bufs=10) as pp,
            tc.tile_pool(name="osb", bufs=3) as op,
            tc.tile_pool(name="py", bufs=3, space="PSUM") as pyp,
            tc.tile_pool(name="pv", bufs=2, space="PSUM") as pvp,
        ):
            # resident loads split across both HWDGE queues, ordered so the
            # first unit's operands land first:
            #   sync: x0[first chunk] -> m1 -> x0[rest] -> x1
            #   ACT : g in 4 chunks
            gcols = DLOC * 8 * 2 * 128
            gq = gcols // 4
            g_sb = rp.tile([128, gcols], bf16, tag="g")
            for qq in range(4):
                nc.scalar.dma_start(out=g_sb[:, qq * gq:(qq + 1) * gq],
                                    in_=g[:, qq * gq:(qq + 1) * gq])
            xt_sb = []
            cut = 514                            # c4 cols for chunk wc=0
            for b in range(NB):
                # SBUF layout [128, ph*NP4 + c4]
                xb = rp.tile([128, XCOLS], bf16, tag=f"x{b}")
                xbr = xb[:, :].rearrange("p (ph c) -> p ph c", ph=4)
                if b == 0:
                    nc.sync.dma_start(out=xbr[:, :, 0:cut],
                                      in_=xT[b, :, :, 0:cut])
                    m1_sb = rp.tile([128, 4 * 2 * 2 * 2 * 128], bf16, tag="m1")
                    nc.sync.dma_start(out=m1_sb[:, :], in_=m1[:, :])
                else:
                    nc.sync.dma_start(out=xbr[:, :, 0:cut],
                                      in_=xT[b, :, :, 0:cut])
                nc.sync.dma_start(out=xbr[:, :, cut:], in_=xT[b, :, :, cut:])
                xt_sb.append(xb)

            def emit_interp_tile(d2, b, w0, wt, n2, t4):
                """One radix y-tile via 8 col-tiled matmuls (4 strips x 2
                source slots), evacuated to SBUF bf16 by ACT."""
                py = pyp.tile([128, WT], f32, tag="py")
                for s in range(2):
                    for h in range(4):
                        m = 4 * t4 + h
                        mp = m + s
                        col = (mp & 3) * NP4 + w0 + (mp >> 2)
                        nc.tensor.matmul(
                            py[32 * h:32 * h + 32, 0:wt],
                            g_sb[:, gcol(d2, m, s, n2)],
                            xt_sb[b][:, col:col + wt],
                            start=(s == 0), stop=(s == 1),
                            tile_position=(0, 32 * h))
                ysb = yp.tile([128, WT], bf16, tag="y")
                nc.scalar.copy(ysb[:, 0:wt], py[:, 0:wt])
                return ysb

            def emit_s1_group(d2, b, w0, wt, n2, mc, ysbs):
                """Stage-1 (n2, mc): 4 matmuls into a [128, re|im] PSUM pair
                + one paired evac (ACT or DVE per knob)."""
                pv = pvp.tile([128, 2 * WT], f32, tag="pv")
                for pl in range(2):
                    for kt in range(2):
                        nc.tensor.matmul(
                            pv[:, pl * WT:pl * WT + wt],
                            m1_sb[:, m1col(n2, pl, kt, mc)],
                            ysbs[(n2, kt)][:, 0:wt],
                            start=(kt == 0), stop=(kt == 1))
                if _VARIANT == "notail":
                    return None
                sv = sp.tile([128, 2 * WT], bf16, tag="sv")
                eng = nc.scalar if _SV_ENG[(mc, n2)] == "A" else nc.vector
                if _SV_ENG[(mc, n2)] == "A":
                    eng.copy(sv[:, :], pv[:, :])
                else:
                    eng.tensor_copy(sv[:, :], pv[:, :])
                return sv

            def emit_tail(d2, b, w0, wt, mc, svs):
                """Partials (DVE, re|im paired) + radix-4 combine
                (DVE for j=0,2 pairs; Pool for j=1,3 halves) + out DMAs."""
                if _VARIANT == "notail":
                    return
                wn = min(wt, W - w0)
                ap = pp.tile([128, 2 * WT], bf16, tag="ap")
                nc.vector.tensor_add(ap[:, :], svs[0][:, :], svs[2][:, :])
                am = pp.tile([128, 2 * WT], bf16, tag="am")
                nc.vector.tensor_sub(am[:, :], svs[0][:, :], svs[2][:, :])
                bp = pp.tile([128, 2 * WT], bf16, tag="bp")
                nc.vector.tensor_add(bp[:, :], svs[1][:, :], svs[3][:, :])
                bm = pp.tile([128, 2 * WT], bf16, tag="bm")
                nc.vector.tensor_sub(bm[:, :], svs[1][:, :], svs[3][:, :])
                # ot col = (j*2 + ri)*WT + w
                ot = op.tile([128, 8 * WT], bf16, tag="ot")
                j02 = nc.vector if _J02_ENG == "V" else nc.gpsimd
                j13 = nc.gpsimd if _J13_ENG == "P" else nc.vector
                # Y0 = A+ + B+ (re|im), Y2 = A+ - B+ (re|im)
                j02.tensor_add(ot[:, 0:2 * WT], ap[:, :], bp[:, :])
                j02.tensor_sub(ot[:, 4 * WT:6 * WT], ap[:, :], bp[:, :])
                # Y1 = A- - iB-: re = am_re + bm_im, im = am_im - bm_re
                j13.tensor_add(ot[:, 2 * WT:3 * WT], am[:, 0:WT],
                               bm[:, WT:2 * WT])
                j13.tensor_sub(ot[:, 3 * WT:4 * WT], am[:, WT:2 * WT],
                               bm[:, 0:WT])
                # Y3 = A- + iB-: re = am_re - bm_im, im = am_im + bm_re
                j13.tensor_sub(ot[:, 6 * WT:7 * WT], am[:, 0:WT],
                               bm[:, WT:2 * WT])
                j13.tensor_add(ot[:, 7 * WT:8 * WT], am[:, WT:2 * WT],
                               bm[:, 0:WT])
                if _VARIANT == "nodma":
                    return
                otr = ot[:, :].rearrange("p (j ri w) -> p ri j w", ri=2, j=4)
                for pl, dram in ((0, out_re), (1, out_im)):
                    nc.sync.dma_start(
                        out=dram[d2, b, mc, :, :, w0:w0 + wn],
                        in_=otr[:, pl, :, 0:wn])

            import contextlib
            import os as _os
            _hints = ()
            if _os.environ.get("LOOP_HINTS"):
                _hints = (mybir.EngineType.PE, mybir.EngineType.Activation,
                          mybir.EngineType.DVE, mybir.EngineType.Pool,
                          mybir.EngineType.SP)
            rep_ctx = (tc.For_i(0, _REPEAT, 1, hint_engines=_hints)
                       if _REPEAT > 1 else contextlib.nullcontext())
            with rep_ctx:
                units = [(d2, b, WT * wc, WT) for d2 in range(DLOC)
                         for b in range(NB) for wc in range(NWC)]
                tiles8 = [(n2, t4) for t4 in range(2) for n2 in range(4)]
                groups8 = [(n2, mc) for mc in range(2) for n2 in range(4)]
                if _VARIANT == "interponly":
                    for it in units:
                        for (n2, t4) in tiles8:
                            emit_interp_tile(*it, n2, t4)
                else:
                    # software pipeline: unit i's stage-1/tail interleaved
                    # with unit i+1's interp tiles (1:1 with the 8 groups)
                    ycur = {(n2, t4): emit_interp_tile(*units[0], n2, t4)
                            for (n2, t4) in tiles8}
                    for i, it in enumerate(units):
                        nxt = units[i + 1] if i + 1 < len(units) else None
                        ynxt = {}
                        svs = {}
                        for gi, (n2, mc) in enumerate(groups8):
                            svs[n2] = emit_s1_group(*it, n2, mc, ycur)
                            if nxt is not None:
                                n2t, t4t = tiles8[gi]
                                ynxt[(n2t, t4t)] = emit_interp_tile(
                                    *nxt, n2t, t4t)
                            if n2 == 3:
                                emit_tail(*it, mc, svs)
                                svs = {}
                        ycur = ynxt
    nc.compile()
    return nc


def _host_prep(x, dlnf):
    x = np.ascontiguousarray(np.asarray(x, dtype=np.float32))
    dlnf = np.asarray(dlnf, dtype=np.float32)
    # x shifted by -64 into partition-interleaved, phase-split layout:
    # xT[b, q, ph, c4] = x[b, 128*(4*c4 + ph) + q - 64]  (zeros outside [0,NX))
    xs = np.zeros((NB, XCOLS * 128), dtype=np.float32)
    xs[:, 64:64 + NX] = x
    xT = np.transpose(xs.reshape(NB, XCOLS, 128), (0, 2, 1))  # [b, q, c]
    xT = xT.reshape(NB, 128, XCOLS // 4, 4).transpose(0, 1, 3, 2)
    xT = np.ascontiguousarray(xT).astype(ml_dtypes.bfloat16)
    m1 = _build_m1()
    lo_all, frac_all = _host_tables_all(dlnf)
    in_maps = []
    for c in range(NCORES):
        gc_ = _build_g(lo_all[DLOC * c: DLOC * (c + 1)],
                       frac_all[DLOC * c: DLOC * (c + 1)])
        in_maps.append({"xT": xT, "g": gc_, "m1": m1})
    return in_maps


def kernel(x, dlnf):
    from concourse.bass_utils import run_bass_kernel_spmd

    in_maps = _host_prep(x, dlnf)
    if "nc" not in _NC_CACHE:
        _NC_CACHE["nc"] = _build_program()
    nc = _NC_CACHE["nc"]
    res = run_bass_kernel_spmd(nc, in_maps, core_ids=list(range(NCORES)))
    _LAST_RESULTS["res"] = res
    outs = []
    for c in range(NCORES):
        # [DLOC, NB, mc, p, j, W] -> k = 256 j + 128 mc + p
        cplx = np.empty((DLOC, NB, K, W), dtype=np.complex64)
        for part, o in (
            (cplx.real, res.results[c]["out_re"]),
            (cplx.imag, res.results[c]["out_im"]),
        ):
            part[...] = np.transpose(
                o.astype(np.float32), (0, 1, 4, 2, 3, 5)).reshape(
                    DLOC, NB, K, W)
        outs.append(np.transpose(cplx, (0, 1, 3, 2)))       # -> [DLOC,NB,W,K]
    return np.concatenate(outs, axis=0)
